# revision 33
# baseline (speedup 1.0000x reference)
"""Trainium2 Bass kernel for a 2-layer GRU (B=4096, T=128, D=32, H=64) + linear head.

Strategy
--------
Data-parallel over batch: B=4096 -> 8 NeuronCores x 512. Each core runs the
full T=128 recurrence for its batch shard. Layout is gate-major: activations
live as [gates/hidden on partitions, batch on the free dim].

The two layers run as a wavefront (layer 1 one step behind layer 0) with the
two hidden states STACKED in one tile S = [g (0:64) ; h (64:128)], which lets
most per-step work be emitted as single 128-partition instructions:

PE (7 matmuls / stage, the packing floor for this shape):
  pzr0  = Wzr0x^T x_s {start} + Wzr0h^T g {stop}        [z0 | r0]
  pzr1  = Wzr1^T  S {start,stop}                        [r1 | z1]
  PH    = Whn^T   S {start,stop}                        [hn1 | hn0]
  PN    = Wn0x^T x_s -> [0:64]{start}, Wn1x^T g -> [64:128]{start}
  PN   += identswap @ T {stop}    (adds t0 -> [0:64], t1 -> [64:128])
ACT (3): rz0 = sigmoid(pzr0+b), rz1 = sigmoid(pzr1+b), n = tanh(PN + bni)
DVE:  t0/t1 = (PH+bnh)*r (stt), zc = 1-z (tensor_scalar 4x),
      u = n*zc, S' = u + w   (both [128,512] stacked across layers)
Pool: w = z * S_prev (both halves, off the critical chain)

Gate order: layer0 [z|r], layer1 [r|z] (mirrored) so every elementwise op has
operands at equal start partitions and the n-chain halves interleave into
single full-width instructions.
"""

import sys

if "/opt/trn_rl_repo" not in sys.path:
    sys.path.insert(0, "/opt/trn_rl_repo")

import numpy as np
import ml_dtypes

B, T, D, H = 4096, 128, 32, 64
NCORES = 8
BL = B // NCORES  # per-core batch = 512

_CACHE = {}


def _legalize_sync(nc, mybir):
    """Split per-instruction semaphore waits that exceed the ISA wait-slot
    budget into EventSemaphore instructions on the same engine queue."""
    budget = {}  # every instruction type: 1 wait max (walrus adds internal waits)
    ctr = 0
    for f in nc.m.functions:
        for blk in f.blocks:
            out = []
            changed = False
            for inst in blk.instructions:
                si = inst.sync_info
                waits = list(si.on_wait) if (si is not None and si.on_wait) else []
                b = budget.get(type(inst).__name__, 1)
                if len(waits) > b:
                    excess, keep = waits[:-b], waits[-b:]
                    for w in excess:
                        ctr += 1
                        out.append(
                            mybir.InstEventSemaphore(
                                name=f"evw{ctr}_{inst.name}",
                                engine=inst.engine,
                                ins=[],
                                outs=[],
                                sync_info=mybir.SyncInfo(on_wait=[w], on_update=[]),
                            )
                        )
                    si.on_wait = keep
                    changed = True
                out.append(inst)
            if changed:
                try:
                    blk.instructions = out
                except Exception:
                    blk.instructions.clear()
                    blk.instructions.extend(out)
    return ctr


def build_module(t_steps=T, bl=BL, reps=1):
    """Build the Bass module (single program, run SPMD on 8 cores)."""
    from contextlib import ExitStack

    import concourse.bass as bass
    import concourse.tile as tile
    from concourse import mybir

    f32 = mybir.dt.float32
    bf16 = mybir.dt.bfloat16
    AF = mybir.ActivationFunctionType
    OP = mybir.AluOpType

    nc = bass.Bass()

    CW = 776  # bf16 const pack width
    x_d = nc.dram_tensor("x", [D, t_steps, bl], bf16, kind="ExternalInput")
    cb_d = nc.dram_tensor("cb", [128, CW], bf16, kind="ExternalInput")
    cf_d = nc.dram_tensor("cf", [128, 8], f32, kind="ExternalInput")
    out_d = nc.dram_tensor("out", [1, bl], f32, kind="ExternalOutput")

    with ExitStack() as ctx:
        tc = ctx.enter_context(tile.TileContext(nc))
        const = ctx.enter_context(tc.tile_pool(name="const", bufs=1))
        spool = ctx.enter_context(tc.tile_pool(name="state", bufs=3))
        work = ctx.enter_context(tc.tile_pool(name="work", bufs=3))
        ps_zr0 = ctx.enter_context(tc.tile_pool(name="ps_zr0", bufs=2, space="PSUM"))
        ps_zr1 = ctx.enter_context(tc.tile_pool(name="ps_zr1", bufs=1, space="PSUM"))
        ps_h = ctx.enter_context(tc.tile_pool(name="ps_h", bufs=1, space="PSUM"))
        ps_n = ctx.enter_context(tc.tile_pool(name="ps_n", bufs=2, space="PSUM"))

        # ---- constants in SBUF (two packed tiles, two DMAs) ----
        cb = const.tile([128, CW], bf16, tag="cb")
        nc.sync.dma_start(out=cb, in_=cb_d[:])
        cf = const.tile([128, 8], f32, tag="cf")
        nc.sync.dma_start(out=cf, in_=cf_d[:])

        wzr0x = cb[0:D, 0:128]
        wzr0h = cb[0:H, 128:256]
        wzr1 = cb[:, 256:384]
        whn = cb[:, 384:512]
        wn0x = cb[0:D, 512:576]
        wn1x = cb[0:H, 576:640]
        idsw = cb[:, 640:768]
        fcw = cb[H:128, 768:769]

        bzr0 = cf[:, 0:1]
        bzr1 = cf[:, 1:2]
        bni = cf[:, 2:3]
        bnh = cf[:, 3:4]  # [bnh1 (0:64) ; bnh0 (64:128)] matching PH layout
        fcb = cf[0:1, 5:6]
        bzneg = cf[:, 6:7]  # negated z-gate biases, for zc = sigmoid(-x)

        # ACT warm-up: absorbs the sigmoid/tanh table-load and the cf DMA
        # wait into an instruction with spare wait slots.
        warm = work.tile([128, 8], f32, tag="warm", bufs=1)
        nc.scalar.activation(warm, cf, AF.Sigmoid)
        warm_v = work.tile([128, 8], f32, tag="warm_v", bufs=1)
        nc.vector.tensor_copy(warm_v, cf)

        # Preload all of x: 8 chunk tiles written once each.
        CH = max(1, t_steps // 8)
        x_chunks = []
        for c in range(0, t_steps, CH):
            n_t = min(CH, t_steps - c)
            xc = const.tile([D, n_t, bl], bf16, tag=f"xc{c}")
            nc.sync.dma_start(out=xc, in_=x_d[:, c : c + n_t, :])
            x_chunks.append(xc)

        def x_slice(s):
            return x_chunks[s // CH][:, s % CH, :]

        S = spool.tile([128, bl], bf16, tag="S")
        nc.vector.memset(S, 0.0)

        n_steps = t_steps * reps

        hb = bl // 2  # column-split point for chain pipelining

        def emit_xpart(s):
            """Allocate stage-s psum tiles and emit its x-only matmuls.

            These have no dependency on the recurrence; with bufs=2 pools
            their slot-reuse WAR naturally delays them into PE's idle
            windows of stage s-2. pn is split into two half-bank tiles so
            the a/b column-half tail chains have no false cross-deps.
            """
            do0 = s < n_steps
            pzr0 = (
                ps_zr0.tile([128, bl], f32, tag="zr0", name=f"pzr0_{s}")
                if do0
                else None
            )
            pn_a = ps_n.tile([128, hb], f32, tag="na", name=f"pna_{s}")
            pn_b = ps_n.tile([128, hb], f32, tag="nb", name=f"pnb_{s}")
            if do0:
                xs = x_slice(s % t_steps)
                nc.tensor.matmul(pzr0, lhsT=wzr0x, rhs=xs, start=True, stop=False)
                nc.tensor.matmul(pn_a[0:H, :], lhsT=wn0x, rhs=xs[:, 0:hb],
                                 start=True, stop=False, skip_group_check=True)
                nc.tensor.matmul(pn_b[0:H, :], lhsT=wn0x, rhs=xs[:, hb:bl],
                                 start=True, stop=False, skip_group_check=True)
            return pzr0, (pn_a, pn_b)

        def stage(s, do0, do1, cur):
            """Emit one wavefront stage; returns the new state tile."""
            nonlocal S
            pzr0, (pn_a, pn_b) = cur

            # ---- PE: state-dependent gate matmuls. pzr1 first (sigma1
            # opens the chain), PH second (t1 reads it directly). ----
            pzr1 = None
            if do1:
                pzr1 = ps_zr1.tile([128, bl], f32, tag="zr1")
                nc.tensor.matmul(pzr1, lhsT=wzr1, rhs=S, start=True, stop=True)
            ph = ps_h.tile([128, bl], f32, tag="hn")
            if do0 and do1:
                nc.tensor.matmul(ph, lhsT=whn, rhs=S, start=True, stop=True)
            elif do0:
                nc.tensor.matmul(ph[H:128, :], lhsT=whn[0:H, 64:128],
                                 rhs=S[0:H, :], start=True, stop=True)
            else:
                nc.tensor.matmul(ph[0:H, :], lhsT=whn[H:128, 0:64],
                                 rhs=S[H:128, :], start=True, stop=True)
            if do0:
                nc.tensor.matmul(pzr0, lhsT=wzr0h, rhs=S[0:H, :], start=False,
                                 stop=True)
            if do1:
                nc.tensor.matmul(pn_a[H:128, :], lhsT=wn1x, rhs=S[0:H, 0:hb],
                                 start=True, stop=False, skip_group_check=True)
                nc.tensor.matmul(pn_b[H:128, :], lhsT=wn1x, rhs=S[0:H, hb:bl],
                                 start=True, stop=False, skip_group_check=True)

            # ---- ACT: sigma1 full, then sigma0 in column halves so t0a
            # can start early ----
            rz0 = rz1 = None
            if do1:
                rz1 = work.tile([128, bl], bf16, tag="rz1")
                nc.scalar.activation(rz1, pzr1, AF.Sigmoid, bias=bzr1)
            if do0:
                rz0 = work.tile([128, bl], bf16, tag="rz0")
                nc.scalar.activation(rz0[:, 0:hb], pzr0[:, 0:hb], AF.Sigmoid,
                                     bias=bzr0)
                nc.scalar.activation(rz0[:, hb:bl], pzr0[:, hb:bl], AF.Sigmoid,
                                     bias=bzr0)

            # t = (hn + bnh) * r, both on DVE (t1 full first, t0 in column
            # halves so ident/tanh column-halves can start early).
            tt = work.tile([128, bl], bf16, tag="t")
            zc = work.tile([128, bl], bf16, tag="zc")
            w = work.tile([128, bl], bf16, tag="w")
            if do1:
                nc.vector.scalar_tensor_tensor(
                    out=tt[0:H, :], in0=ph[0:H, :], scalar=bnh[0:H, :],
                    in1=rz1[0:H, :], op0=OP.add, op1=OP.mult)
                nc.gpsimd.tensor_mul(w[H:128, :], rz1[H:128, :], S[H:128, :])
            if do0:
                if do1:
                    nc.vector.scalar_tensor_tensor(
                        out=tt[H:128, 0:hb], in0=ph[H:128, 0:hb],
                        scalar=bnh[H:128, :], in1=rz0[H:128, 0:hb],
                        op0=OP.add, op1=OP.mult)
                    nc.vector.scalar_tensor_tensor(
                        out=tt[H:128, hb:bl], in0=ph[H:128, hb:bl],
                        scalar=bnh[H:128, :], in1=rz0[H:128, hb:bl],
                        op0=OP.add, op1=OP.mult)
                else:
                    nc.vector.scalar_tensor_tensor(
                        out=tt[H:128, :], in0=ph[H:128, :],
                        scalar=bnh[H:128, :], in1=rz0[H:128, :],
                        op0=OP.add, op1=OP.mult)
                # zc0 on DVE (after the t-block): keeps ACT free for tanhs
                nc.vector.tensor_scalar(
                    out=zc[0:H, :], in0=rz0[0:H, :], scalar1=-1.0, scalar2=1.0,
                    op0=OP.mult, op1=OP.add)
                nc.gpsimd.tensor_mul(w[0:H, :], rz0[0:H, :], S[0:H, :])
            if do1:
                # zc1 on ACT, emitted here so it sits between sigma0 and the
                # tanh halves in ACT's queue
                nc.scalar.activation(zc[H:128, :], rz1[H:128, :], AF.Copy,
                                     bias=1.0, scale=-1.0)

            # ---- PE ident + ACT tanh + DVE u/ns, pipelined in column
            # halves so the tail overlaps itself ----
            nn = work.tile([128, bl], bf16, tag="n")
            u = work.tile([128, bl], bf16, tag="u")
            S_new = spool.tile([128, bl], bf16, tag="S")
            lo, hi = (0 if do0 else H), (128 if do1 else H)

            if do0 and do1:
                for pnh, c0, c1 in ((pn_a, 0, hb), (pn_b, hb, bl)):
                    nc.tensor.matmul(pnh, lhsT=idsw, rhs=tt[:, c0:c1],
                                     start=False, stop=True,
                                     skip_group_check=True)
                    nc.scalar.activation(nn[:, c0:c1], pnh, AF.Tanh, bias=bni)
                    nc.vector.tensor_mul(u[:, c0:c1], nn[:, c0:c1],
                                         zc[:, c0:c1])
                    nc.vector.tensor_add(S_new[:, c0:c1], u[:, c0:c1],
                                         w[:, c0:c1])
            else:
                for pnh, c0, c1 in ((pn_a, 0, hb), (pn_b, hb, bl)):
                    if do0:
                        nc.tensor.matmul(pnh[0:H, :], lhsT=idsw[H:128, 0:64],
                                         rhs=tt[H:128, c0:c1], start=False,
                                         stop=True, skip_group_check=True)
                    else:
                        nc.tensor.matmul(pnh[H:128, :], lhsT=idsw[0:H, 64:128],
                                         rhs=tt[0:H, c0:c1], start=False,
                                         stop=True, skip_group_check=True)
                    nc.scalar.activation(nn[lo:hi, c0:c1], pnh[lo:hi, :],
                                         AF.Tanh, bias=bni[lo:hi, :])
                    nc.vector.tensor_mul(u[lo:hi, c0:c1], nn[lo:hi, c0:c1],
                                         zc[lo:hi, c0:c1])
                    nc.vector.tensor_add(S_new[lo:hi, c0:c1], u[lo:hi, c0:c1],
                                         w[lo:hi, c0:c1])
                olo, ohi = (H, 128) if do0 else (0, H)
                nc.vector.tensor_copy(S_new[olo:ohi, :], S[olo:ohi, :])
            S = S_new

            # Emit stage s+2's x-matmuls at the stage BOTTOM. With bufs=2 on
            # their psum pools, slot-reuse WAR naturally delays them: the
            # zr x-matmul becomes ready when sigma0(s) frees its slot (PE's
            # idle sigma-window) and the n x-matmul when tanh(s) does (PE's
            # idle tail window) — so neither bypasses in front of the
            # chain-critical ident matmuls.
            if s + 2 <= n_steps:
                tiles[s + 2] = emit_xpart(s + 2)

        tiles = {0: emit_xpart(0), 1: emit_xpart(1)}
        for s in range(n_steps + 1):
            stage(s, do0=(s < n_steps), do1=(s >= 1), cur=tiles.pop(s))

        # final projection: out = fc_w @ h + fc_b   -> [1, bl]
        pfc = ps_zr0.tile([1, bl], f32, tag="zr0")
        nc.tensor.matmul(pfc, lhsT=fcw, rhs=S[H:128, :], start=True, stop=True)
        out_sb = work.tile([1, bl], f32, tag="out", bufs=1)
        nc.scalar.activation(out_sb, pfc, AF.Identity, bias=fcb)
        nc.sync.dma_start(out=out_d[:], in_=out_sb)

    _legalize_sync(nc, mybir)
    return nc


def shard_inputs(inputs, bl=BL, ncores=NCORES, t_steps=T):
    """Host-side prep: transpose/cast/shard full inputs into per-core maps."""
    bf = ml_dtypes.bfloat16
    x = np.asarray(inputs["x"], dtype=np.float32)
    # [B, T, D] -> [D, T, B]
    xT = np.ascontiguousarray(
        x[: bl * ncores, :t_steps, :].transpose(2, 1, 0)
    ).astype(bf)

    f32 = np.float32
    Wih0 = np.asarray(inputs["W_ih0"], f32)
    Whh0 = np.asarray(inputs["W_hh0"], f32)
    Wih1 = np.asarray(inputs["W_ih1"], f32)
    Whh1 = np.asarray(inputs["W_hh1"], f32)

    CW = 776
    cb = np.zeros((128, CW), dtype=bf)
    # layer0 gate order [z | r], layer1 mirrored [r | z]
    cb[0:D, 0:128] = np.concatenate([Wih0[H : 2 * H], Wih0[0:H]], axis=0).T.astype(bf)
    cb[0:H, 128:256] = np.concatenate([Whh0[H : 2 * H], Whh0[0:H]], axis=0).T.astype(bf)
    wzr1 = np.zeros((128, 128), f32)
    wzr1[0:H, :] = np.concatenate([Wih1[0:H], Wih1[H : 2 * H]], axis=0).T
    wzr1[H:128, :] = np.concatenate([Whh1[0:H], Whh1[H : 2 * H]], axis=0).T
    cb[:, 256:384] = wzr1.astype(bf)
    whn = np.zeros((128, 128), f32)
    whn[H:128, 0:H] = Whh1[2 * H :].T  # hn1 from h
    whn[0:H, H:128] = Whh0[2 * H :].T  # hn0 from g
    cb[:, 384:512] = whn.astype(bf)
    cb[0:D, 512:576] = Wih0[2 * H :].T.astype(bf)
    cb[0:H, 576:640] = Wih1[2 * H :].T.astype(bf)
    idsw = np.zeros((128, 128), f32)
    idsw[H:128, 0:H] = np.eye(H)  # out[0:64]  <- T[64:128]
    idsw[0:H, H:128] = np.eye(H)  # out[64:128] <- T[0:64]
    cb[:, 640:768] = idsw.astype(bf)
    cb[H:128, 768] = np.asarray(inputs["fc_w"], f32).reshape(H).astype(bf)

    cf = np.zeros((128, 8), dtype=f32)
    b0 = np.asarray(inputs["b_ih0"], f32) + np.asarray(inputs["b_hh0"], f32)
    b1 = np.asarray(inputs["b_ih1"], f32) + np.asarray(inputs["b_hh1"], f32)
    cf[:, 0] = np.concatenate([b0[H : 2 * H], b0[0:H]])  # [z0 | r0]
    cf[:, 1] = np.concatenate([b1[0:H], b1[H : 2 * H]])  # [r1 | z1]
    cf[0:H, 2] = np.asarray(inputs["b_ih0"], f32)[2 * H :]
    cf[H:128, 2] = np.asarray(inputs["b_ih1"], f32)[2 * H :]
    cf[0:H, 3] = np.asarray(inputs["b_hh1"], f32)[2 * H :]
    cf[H:128, 3] = np.asarray(inputs["b_hh0"], f32)[2 * H :]
    cf[0, 5] = np.asarray(inputs["fc_b"], f32).reshape(())
    cf[0:H, 6] = -b0[H : 2 * H]  # -z0 bias
    cf[H:128, 6] = -b1[H : 2 * H]  # -z1 bias

    shared = {"cb": cb, "cf": cf}

    in_maps = []
    for c in range(ncores):
        m = dict(shared)
        m["x"] = np.ascontiguousarray(xT[:, :, c * bl : (c + 1) * bl])
        in_maps.append(m)
    return in_maps


def kernel(**inputs):
    from concourse import bass_utils

    if "nc" not in _CACHE:
        _CACHE["nc"] = build_module()
    nc = _CACHE["nc"]
    in_maps = shard_inputs(inputs)
    res = bass_utils.run_bass_kernel_spmd(nc, in_maps, core_ids=list(range(NCORES)))
    out = np.concatenate([r["out"].reshape(BL) for r in res.results])
    return out.astype(np.float32)


# revision 34
# speedup vs baseline: 1.1365x; 1.1365x over previous
"""Trainium2 Bass kernel for a 2-layer GRU (B=4096, T=128, D=32, H=64) + linear head.

Strategy
--------
Data-parallel over batch: B=4096 -> 8 NeuronCores x 512. Each core runs the
full T=128 recurrence for its batch shard. Layout is gate-major: activations
live as [gates/hidden on partitions, batch on the free dim].

The two layers run as a wavefront (layer 1 one step behind layer 0) with the
two hidden states STACKED in one tile S = [g (0:64) ; h (64:128)], which lets
most per-step work be emitted as single 128-partition instructions:

PE (7 matmuls / stage, the packing floor for this shape):
  pzr0  = Wzr0x^T x_s {start} + Wzr0h^T g {stop}        [z0 | r0]
  pzr1  = Wzr1^T  S {start,stop}                        [r1 | z1]
  PH    = Whn^T   S {start,stop}                        [hn1 | hn0]
  PN    = Wn0x^T x_s -> [0:64]{start}, Wn1x^T g -> [64:128]{start}
  PN   += identswap @ T {stop}    (adds t0 -> [0:64], t1 -> [64:128])
ACT (3): rz0 = sigmoid(pzr0+b), rz1 = sigmoid(pzr1+b), n = tanh(PN + bni)
DVE:  t0/t1 = (PH+bnh)*r (stt), zc = 1-z (tensor_scalar 4x),
      u = n*zc, S' = u + w   (both [128,512] stacked across layers)
Pool: w = z * S_prev (both halves, off the critical chain)

Gate order: layer0 [z|r], layer1 [r|z] (mirrored) so every elementwise op has
operands at equal start partitions and the n-chain halves interleave into
single full-width instructions.
"""

import sys

if "/opt/trn_rl_repo" not in sys.path:
    sys.path.insert(0, "/opt/trn_rl_repo")

import numpy as np
import ml_dtypes

B, T, D, H = 4096, 128, 32, 64
NCORES = 8
BL = B // NCORES  # per-core batch = 512

_CACHE = {}


def _legalize_sync(nc, mybir):
    """Split per-instruction semaphore waits that exceed the ISA wait-slot
    budget into EventSemaphore instructions on the same engine queue."""
    budget = {}  # every instruction type: 1 wait max (walrus adds internal waits)
    ctr = 0
    for f in nc.m.functions:
        for blk in f.blocks:
            out = []
            changed = False
            for inst in blk.instructions:
                si = inst.sync_info
                waits = list(si.on_wait) if (si is not None and si.on_wait) else []
                b = budget.get(type(inst).__name__, 1)
                if len(waits) > b:
                    excess, keep = waits[:-b], waits[-b:]
                    for w in excess:
                        ctr += 1
                        out.append(
                            mybir.InstEventSemaphore(
                                name=f"evw{ctr}_{inst.name}",
                                engine=inst.engine,
                                ins=[],
                                outs=[],
                                sync_info=mybir.SyncInfo(on_wait=[w], on_update=[]),
                            )
                        )
                    si.on_wait = keep
                    changed = True
                out.append(inst)
            if changed:
                try:
                    blk.instructions = out
                except Exception:
                    blk.instructions.clear()
                    blk.instructions.extend(out)
    return ctr


def build_module(t_steps=T, bl=BL, reps=1):
    """Build the Bass module (single program, run SPMD on 8 cores)."""
    from contextlib import ExitStack

    import concourse.bass as bass
    import concourse.tile as tile
    from concourse import mybir

    f32 = mybir.dt.float32
    bf16 = mybir.dt.bfloat16
    AF = mybir.ActivationFunctionType
    OP = mybir.AluOpType

    nc = bass.Bass()

    CW = 776  # bf16 const pack width
    x_d = nc.dram_tensor("x", [D, t_steps, bl], bf16, kind="ExternalInput")
    cb_d = nc.dram_tensor("cb", [128, CW], bf16, kind="ExternalInput")
    cf_d = nc.dram_tensor("cf", [128, 8], f32, kind="ExternalInput")
    out_d = nc.dram_tensor("out", [1, bl], f32, kind="ExternalOutput")

    with ExitStack() as ctx:
        tc = ctx.enter_context(tile.TileContext(nc))
        const = ctx.enter_context(tc.tile_pool(name="const", bufs=1))
        spool = ctx.enter_context(tc.tile_pool(name="state", bufs=3))
        work = ctx.enter_context(tc.tile_pool(name="work", bufs=3))
        ps_zr0 = ctx.enter_context(tc.tile_pool(name="ps_zr0", bufs=2, space="PSUM"))
        ps_zr1 = ctx.enter_context(tc.tile_pool(name="ps_zr1", bufs=1, space="PSUM"))
        ps_h = ctx.enter_context(tc.tile_pool(name="ps_h", bufs=1, space="PSUM"))
        ps_n = ctx.enter_context(tc.tile_pool(name="ps_n", bufs=2, space="PSUM"))

        # ---- constants in SBUF (two packed tiles, two DMAs) ----
        cb = const.tile([128, CW], bf16, tag="cb")
        nc.sync.dma_start(out=cb, in_=cb_d[:])
        cf = const.tile([128, 8], f32, tag="cf")
        nc.sync.dma_start(out=cf, in_=cf_d[:])

        wzr0x = cb[0:D, 0:128]
        wzr0h = cb[0:H, 128:256]
        wzr1 = cb[:, 256:384]
        whn = cb[:, 384:512]
        wn0x = cb[0:D, 512:576]
        wn1x = cb[0:H, 576:640]
        idsw = cb[:, 640:768]
        fcw = cb[H:128, 768:769]

        bzr0 = cf[:, 0:1]
        bzr1 = cf[:, 1:2]
        bni = cf[:, 2:3]
        bnh = cf[:, 3:4]  # [bnh1 (0:64) ; bnh0 (64:128)] matching PH layout
        fcb = cf[0:1, 5:6]
        bzneg = cf[:, 6:7]  # negated z-gate biases, for zc = sigmoid(-x)

        # ACT warm-up: absorbs the sigmoid/tanh table-load and the cf DMA
        # wait into an instruction with spare wait slots.
        warm = work.tile([128, 8], f32, tag="warm", bufs=1)
        nc.scalar.activation(warm, cf, AF.Sigmoid)
        warm_v = work.tile([128, 8], f32, tag="warm_v", bufs=1)
        nc.vector.tensor_copy(warm_v, cf)

        # Preload all of x: 8 chunk tiles written once each.
        CH = max(1, t_steps // 8)
        x_chunks = []
        for c in range(0, t_steps, CH):
            n_t = min(CH, t_steps - c)
            xc = const.tile([D, n_t, bl], bf16, tag=f"xc{c}")
            nc.sync.dma_start(out=xc, in_=x_d[:, c : c + n_t, :])
            x_chunks.append(xc)

        def x_slice(s):
            return x_chunks[s // CH][:, s % CH, :]

        S = spool.tile([128, bl], bf16, tag="S")
        nc.vector.memset(S, 0.0)

        n_steps = t_steps * reps

        hb = bl // 2  # column-split point for chain pipelining

        def emit_xpart(s):
            """Allocate stage-s psum tiles and emit its x-only matmuls.

            These have no dependency on the recurrence; with bufs=2 pools
            their slot-reuse WAR naturally delays them into PE's idle
            windows of stage s-2. pn is split into two half-bank tiles so
            the a/b column-half tail chains have no false cross-deps.
            """
            do0 = s < n_steps
            pzr0 = (
                ps_zr0.tile([128, bl], f32, tag="zr0", name=f"pzr0_{s}")
                if do0
                else None
            )
            pn_a = ps_n.tile([128, hb], f32, tag="na", name=f"pna_{s}")
            pn_b = ps_n.tile([128, hb], f32, tag="nb", name=f"pnb_{s}")
            if do0:
                xs = x_slice(s % t_steps)
                nc.tensor.matmul(pzr0, lhsT=wzr0x, rhs=xs, start=True, stop=False)
                nc.tensor.matmul(pn_a[0:H, :], lhsT=wn0x, rhs=xs[:, 0:hb],
                                 start=True, stop=False, skip_group_check=True)
                nc.tensor.matmul(pn_b[0:H, :], lhsT=wn0x, rhs=xs[:, hb:bl],
                                 start=True, stop=False, skip_group_check=True)
            return pzr0, (pn_a, pn_b)

        def stage(s, do0, do1, cur):
            """Emit one wavefront stage; returns the new state tile."""
            nonlocal S
            pzr0, (pn_a, pn_b) = cur

            # ---- PE: state-dependent gate matmuls. pzr1 first: sigma1
            # opens the chain (t1 is the long pole through DVE). ----
            pzr1 = None
            if do1:
                pzr1 = ps_zr1.tile([128, bl], f32, tag="zr1")
                nc.tensor.matmul(pzr1, lhsT=wzr1, rhs=S, start=True, stop=True)
            if do0:
                nc.tensor.matmul(pzr0, lhsT=wzr0h, rhs=S[0:H, :], start=False,
                                 stop=True)
            ph = ps_h.tile([128, bl], f32, tag="hn")
            if do0 and do1:
                nc.tensor.matmul(ph, lhsT=whn, rhs=S, start=True, stop=True)
            elif do0:
                nc.tensor.matmul(ph[H:128, :], lhsT=whn[0:H, 64:128],
                                 rhs=S[0:H, :], start=True, stop=True)
            else:
                nc.tensor.matmul(ph[0:H, :], lhsT=whn[H:128, 0:64],
                                 rhs=S[H:128, :], start=True, stop=True)
            if do1:
                nc.tensor.matmul(pn_a[H:128, :], lhsT=wn1x, rhs=S[0:H, 0:hb],
                                 start=True, stop=False, skip_group_check=True)
                nc.tensor.matmul(pn_b[H:128, :], lhsT=wn1x, rhs=S[0:H, hb:bl],
                                 start=True, stop=False, skip_group_check=True)

            # ---- ACT: sigmas (sigma1 first); zc = sigmoid(-x) later fills
            # ACT's idle window before the tanh halves ----
            rz0 = rz1 = None
            if do1:
                rz1 = work.tile([128, bl], bf16, tag="rz1")
                nc.scalar.activation(rz1, pzr1, AF.Sigmoid, bias=bzr1)
            if do0:
                rz0 = work.tile([128, bl], bf16, tag="rz0")
                nc.scalar.activation(rz0, pzr0, AF.Sigmoid, bias=bzr0)

            # t = (hn + bnh) * r, both on DVE (t1 full first, t0 in column
            # halves so ident/tanh column-halves can start early).
            tt = work.tile([128, bl], bf16, tag="t")
            zc = work.tile([128, bl], bf16, tag="zc")
            w = work.tile([128, bl], bf16, tag="w")
            if do1:
                nc.vector.scalar_tensor_tensor(
                    out=tt[0:H, :], in0=ph[0:H, :], scalar=bnh[0:H, :],
                    in1=rz1[0:H, :], op0=OP.add, op1=OP.mult)
                nc.scalar.activation(zc[H:128, :], rz1[H:128, :], AF.Copy,
                                     bias=1.0, scale=-1.0)
                nc.gpsimd.tensor_mul(w[H:128, :], rz1[H:128, :], S[H:128, :])
            if do0:
                if do1:
                    nc.vector.scalar_tensor_tensor(
                        out=tt[H:128, 0:hb], in0=ph[H:128, 0:hb],
                        scalar=bnh[H:128, :], in1=rz0[H:128, 0:hb],
                        op0=OP.add, op1=OP.mult)
                    nc.vector.scalar_tensor_tensor(
                        out=tt[H:128, hb:bl], in0=ph[H:128, hb:bl],
                        scalar=bnh[H:128, :], in1=rz0[H:128, hb:bl],
                        op0=OP.add, op1=OP.mult)
                else:
                    nc.vector.scalar_tensor_tensor(
                        out=tt[H:128, :], in0=ph[H:128, :],
                        scalar=bnh[H:128, :], in1=rz0[H:128, :],
                        op0=OP.add, op1=OP.mult)
                nc.scalar.activation(zc[0:H, :], rz0[0:H, :], AF.Copy,
                                     bias=1.0, scale=-1.0)
                nc.gpsimd.tensor_mul(w[0:H, :], rz0[0:H, :], S[0:H, :])

            # ---- PE ident + ACT tanh + DVE u/ns, pipelined in column
            # halves so the tail overlaps itself ----
            nn = work.tile([128, bl], bf16, tag="n")
            u = work.tile([128, bl], bf16, tag="u")
            S_new = spool.tile([128, bl], bf16, tag="S")
            lo, hi = (0 if do0 else H), (128 if do1 else H)

            if do0 and do1:
                for pnh, c0, c1 in ((pn_a, 0, hb), (pn_b, hb, bl)):
                    nc.tensor.matmul(pnh, lhsT=idsw, rhs=tt[:, c0:c1],
                                     start=False, stop=True,
                                     skip_group_check=True)
                    nc.scalar.activation(nn[:, c0:c1], pnh, AF.Tanh, bias=bni)
                    nc.vector.tensor_mul(u[:, c0:c1], nn[:, c0:c1],
                                         zc[:, c0:c1])
                    nc.vector.tensor_add(S_new[:, c0:c1], u[:, c0:c1],
                                         w[:, c0:c1])
            else:
                for pnh, c0, c1 in ((pn_a, 0, hb), (pn_b, hb, bl)):
                    if do0:
                        nc.tensor.matmul(pnh[0:H, :], lhsT=idsw[H:128, 0:64],
                                         rhs=tt[H:128, c0:c1], start=False,
                                         stop=True, skip_group_check=True)
                    else:
                        nc.tensor.matmul(pnh[H:128, :], lhsT=idsw[0:H, 64:128],
                                         rhs=tt[0:H, c0:c1], start=False,
                                         stop=True, skip_group_check=True)
                    nc.scalar.activation(nn[lo:hi, c0:c1], pnh[lo:hi, :],
                                         AF.Tanh, bias=bni[lo:hi, :])
                    nc.vector.tensor_mul(u[lo:hi, c0:c1], nn[lo:hi, c0:c1],
                                         zc[lo:hi, c0:c1])
                    nc.vector.tensor_add(S_new[lo:hi, c0:c1], u[lo:hi, c0:c1],
                                         w[lo:hi, c0:c1])
                olo, ohi = (H, 128) if do0 else (0, H)
                nc.vector.tensor_copy(S_new[olo:ohi, :], S[olo:ohi, :])
            S = S_new

            # Emit stage s+2's x-matmuls at the stage BOTTOM. With bufs=2 on
            # their psum pools, slot-reuse WAR naturally delays them: the
            # zr x-matmul becomes ready when sigma0(s) frees its slot (PE's
            # idle sigma-window) and the n x-matmul when tanh(s) does (PE's
            # idle tail window) — so neither bypasses in front of the
            # chain-critical ident matmuls.
            if s + 2 <= n_steps:
                tiles[s + 2] = emit_xpart(s + 2)

        tiles = {0: emit_xpart(0), 1: emit_xpart(1)}
        for s in range(n_steps + 1):
            stage(s, do0=(s < n_steps), do1=(s >= 1), cur=tiles.pop(s))

        # final projection: out = fc_w @ h + fc_b   -> [1, bl]
        pfc = ps_zr0.tile([1, bl], f32, tag="zr0")
        nc.tensor.matmul(pfc, lhsT=fcw, rhs=S[H:128, :], start=True, stop=True)
        out_sb = work.tile([1, bl], f32, tag="out", bufs=1)
        nc.scalar.activation(out_sb, pfc, AF.Identity, bias=fcb)
        nc.sync.dma_start(out=out_d[:], in_=out_sb)

    _legalize_sync(nc, mybir)
    return nc


def shard_inputs(inputs, bl=BL, ncores=NCORES, t_steps=T):
    """Host-side prep: transpose/cast/shard full inputs into per-core maps."""
    bf = ml_dtypes.bfloat16
    x = np.asarray(inputs["x"], dtype=np.float32)
    # [B, T, D] -> [D, T, B]
    xT = np.ascontiguousarray(
        x[: bl * ncores, :t_steps, :].transpose(2, 1, 0)
    ).astype(bf)

    f32 = np.float32
    Wih0 = np.asarray(inputs["W_ih0"], f32)
    Whh0 = np.asarray(inputs["W_hh0"], f32)
    Wih1 = np.asarray(inputs["W_ih1"], f32)
    Whh1 = np.asarray(inputs["W_hh1"], f32)

    CW = 776
    cb = np.zeros((128, CW), dtype=bf)
    # layer0 gate order [z | r], layer1 mirrored [r | z]
    cb[0:D, 0:128] = np.concatenate([Wih0[H : 2 * H], Wih0[0:H]], axis=0).T.astype(bf)
    cb[0:H, 128:256] = np.concatenate([Whh0[H : 2 * H], Whh0[0:H]], axis=0).T.astype(bf)
    wzr1 = np.zeros((128, 128), f32)
    wzr1[0:H, :] = np.concatenate([Wih1[0:H], Wih1[H : 2 * H]], axis=0).T
    wzr1[H:128, :] = np.concatenate([Whh1[0:H], Whh1[H : 2 * H]], axis=0).T
    cb[:, 256:384] = wzr1.astype(bf)
    whn = np.zeros((128, 128), f32)
    whn[H:128, 0:H] = Whh1[2 * H :].T  # hn1 from h
    whn[0:H, H:128] = Whh0[2 * H :].T  # hn0 from g
    cb[:, 384:512] = whn.astype(bf)
    cb[0:D, 512:576] = Wih0[2 * H :].T.astype(bf)
    cb[0:H, 576:640] = Wih1[2 * H :].T.astype(bf)
    idsw = np.zeros((128, 128), f32)
    idsw[H:128, 0:H] = np.eye(H)  # out[0:64]  <- T[64:128]
    idsw[0:H, H:128] = np.eye(H)  # out[64:128] <- T[0:64]
    cb[:, 640:768] = idsw.astype(bf)
    cb[H:128, 768] = np.asarray(inputs["fc_w"], f32).reshape(H).astype(bf)

    cf = np.zeros((128, 8), dtype=f32)
    b0 = np.asarray(inputs["b_ih0"], f32) + np.asarray(inputs["b_hh0"], f32)
    b1 = np.asarray(inputs["b_ih1"], f32) + np.asarray(inputs["b_hh1"], f32)
    cf[:, 0] = np.concatenate([b0[H : 2 * H], b0[0:H]])  # [z0 | r0]
    cf[:, 1] = np.concatenate([b1[0:H], b1[H : 2 * H]])  # [r1 | z1]
    cf[0:H, 2] = np.asarray(inputs["b_ih0"], f32)[2 * H :]
    cf[H:128, 2] = np.asarray(inputs["b_ih1"], f32)[2 * H :]
    cf[0:H, 3] = np.asarray(inputs["b_hh1"], f32)[2 * H :]
    cf[H:128, 3] = np.asarray(inputs["b_hh0"], f32)[2 * H :]
    cf[0, 5] = np.asarray(inputs["fc_b"], f32).reshape(())
    cf[0:H, 6] = -b0[H : 2 * H]  # -z0 bias
    cf[H:128, 6] = -b1[H : 2 * H]  # -z1 bias

    shared = {"cb": cb, "cf": cf}

    in_maps = []
    for c in range(ncores):
        m = dict(shared)
        m["x"] = np.ascontiguousarray(xT[:, :, c * bl : (c + 1) * bl])
        in_maps.append(m)
    return in_maps


def kernel(**inputs):
    from concourse import bass_utils

    if "nc" not in _CACHE:
        _CACHE["nc"] = build_module()
    nc = _CACHE["nc"]
    in_maps = shard_inputs(inputs)
    res = bass_utils.run_bass_kernel_spmd(nc, in_maps, core_ids=list(range(NCORES)))
    out = np.concatenate([r["out"].reshape(BL) for r in res.results])
    return out.astype(np.float32)


# revision 35
# speedup vs baseline: 1.1377x; 1.0011x over previous
"""Trainium2 Bass kernel for a 2-layer GRU (B=4096, T=128, D=32, H=64) + linear head.

Strategy
--------
Data-parallel over batch: B=4096 -> 8 NeuronCores x 512. Each core runs the
full T=128 recurrence for its batch shard. Layout is gate-major: activations
live as [gates/hidden on partitions, batch on the free dim].

The two layers run as a wavefront (layer 1 one step behind layer 0) with the
two hidden states STACKED in one tile S = [g (0:64) ; h (64:128)], which lets
most per-step work be emitted as single 128-partition instructions:

PE (7 matmuls / stage, the packing floor for this shape):
  pzr0  = Wzr0x^T x_s {start} + Wzr0h^T g {stop}        [z0 | r0]
  pzr1  = Wzr1^T  S {start,stop}                        [r1 | z1]
  PH    = Whn^T   S {start,stop}                        [hn1 | hn0]
  PN    = Wn0x^T x_s -> [0:64]{start}, Wn1x^T g -> [64:128]{start}
  PN   += identswap @ T {stop}    (adds t0 -> [0:64], t1 -> [64:128])
ACT (3): rz0 = sigmoid(pzr0+b), rz1 = sigmoid(pzr1+b), n = tanh(PN + bni)
DVE:  t0/t1 = (PH+bnh)*r (stt), zc = 1-z (tensor_scalar 4x),
      u = n*zc, S' = u + w   (both [128,512] stacked across layers)
Pool: w = z * S_prev (both halves, off the critical chain)

Gate order: layer0 [z|r], layer1 [r|z] (mirrored) so every elementwise op has
operands at equal start partitions and the n-chain halves interleave into
single full-width instructions.
"""

import sys

if "/opt/trn_rl_repo" not in sys.path:
    sys.path.insert(0, "/opt/trn_rl_repo")

import numpy as np
import ml_dtypes

B, T, D, H = 4096, 128, 32, 64
NCORES = 8
BL = B // NCORES  # per-core batch = 512

_CACHE = {}


def _legalize_sync(nc, mybir):
    """Split per-instruction semaphore waits that exceed the ISA wait-slot
    budget into EventSemaphore instructions on the same engine queue."""
    budget = {}  # every instruction type: 1 wait max (walrus adds internal waits)
    ctr = 0
    for f in nc.m.functions:
        for blk in f.blocks:
            out = []
            changed = False
            for inst in blk.instructions:
                si = inst.sync_info
                waits = list(si.on_wait) if (si is not None and si.on_wait) else []
                b = budget.get(type(inst).__name__, 1)
                if len(waits) > b:
                    excess, keep = waits[:-b], waits[-b:]
                    for w in excess:
                        ctr += 1
                        out.append(
                            mybir.InstEventSemaphore(
                                name=f"evw{ctr}_{inst.name}",
                                engine=inst.engine,
                                ins=[],
                                outs=[],
                                sync_info=mybir.SyncInfo(on_wait=[w], on_update=[]),
                            )
                        )
                    si.on_wait = keep
                    changed = True
                out.append(inst)
            if changed:
                try:
                    blk.instructions = out
                except Exception:
                    blk.instructions.clear()
                    blk.instructions.extend(out)
    return ctr


def build_module(t_steps=T, bl=BL, reps=1):
    """Build the Bass module (single program, run SPMD on 8 cores)."""
    from contextlib import ExitStack

    import concourse.bass as bass
    import concourse.tile as tile
    from concourse import mybir

    f32 = mybir.dt.float32
    bf16 = mybir.dt.bfloat16
    AF = mybir.ActivationFunctionType
    OP = mybir.AluOpType

    nc = bass.Bass()

    CW = 776  # bf16 const pack width
    x_d = nc.dram_tensor("x", [D, t_steps, bl], bf16, kind="ExternalInput")
    cb_d = nc.dram_tensor("cb", [128, CW], bf16, kind="ExternalInput")
    cf_d = nc.dram_tensor("cf", [128, 8], f32, kind="ExternalInput")
    out_d = nc.dram_tensor("out", [1, bl], f32, kind="ExternalOutput")

    with ExitStack() as ctx:
        tc = ctx.enter_context(tile.TileContext(nc))
        const = ctx.enter_context(tc.tile_pool(name="const", bufs=1))
        spool = ctx.enter_context(tc.tile_pool(name="state", bufs=3))
        work = ctx.enter_context(tc.tile_pool(name="work", bufs=3))
        ps_zr0 = ctx.enter_context(tc.tile_pool(name="ps_zr0", bufs=2, space="PSUM"))
        ps_zr1 = ctx.enter_context(tc.tile_pool(name="ps_zr1", bufs=1, space="PSUM"))
        ps_h = ctx.enter_context(tc.tile_pool(name="ps_h", bufs=1, space="PSUM"))
        ps_n = ctx.enter_context(tc.tile_pool(name="ps_n", bufs=2, space="PSUM"))

        # ---- constants in SBUF (two packed tiles, two DMAs) ----
        cb = const.tile([128, CW], bf16, tag="cb")
        nc.sync.dma_start(out=cb, in_=cb_d[:])
        cf = const.tile([128, 8], f32, tag="cf")
        nc.sync.dma_start(out=cf, in_=cf_d[:])

        wzr0x = cb[0:D, 0:128]
        wzr0h = cb[0:H, 128:256]
        wzr1 = cb[:, 256:384]
        whn = cb[:, 384:512]
        wn0x = cb[0:D, 512:576]
        wn1x = cb[0:H, 576:640]
        idsw = cb[:, 640:768]
        fcw = cb[H:128, 768:769]

        bzr0 = cf[:, 0:1]
        bzr1 = cf[:, 1:2]
        bni = cf[:, 2:3]
        bnh = cf[:, 3:4]  # [bnh1 (0:64) ; bnh0 (64:128)] matching PH layout
        fcb = cf[0:1, 5:6]
        bzneg = cf[:, 6:7]  # negated z-gate biases, for zc = sigmoid(-x)

        # ACT warm-up: absorbs the sigmoid/tanh table-load and the cf DMA
        # wait into an instruction with spare wait slots.
        warm = work.tile([128, 8], f32, tag="warm", bufs=1)
        nc.scalar.activation(warm, cf, AF.Sigmoid)
        warm_v = work.tile([128, 8], f32, tag="warm_v", bufs=1)
        nc.vector.tensor_copy(warm_v, cf)

        # Preload all of x: 8 chunk tiles written once each.
        CH = max(1, t_steps // 8)
        x_chunks = []
        for c in range(0, t_steps, CH):
            n_t = min(CH, t_steps - c)
            xc = const.tile([D, n_t, bl], bf16, tag=f"xc{c}")
            nc.sync.dma_start(out=xc, in_=x_d[:, c : c + n_t, :])
            x_chunks.append(xc)

        def x_slice(s):
            return x_chunks[s // CH][:, s % CH, :]

        S = spool.tile([128, bl], bf16, tag="S")
        nc.vector.memset(S, 0.0)

        n_steps = t_steps * reps

        hb = bl // 2  # column-split point for chain pipelining

        def emit_xpart(s):
            """Allocate stage-s psum tiles and emit its x-only matmuls.

            These have no dependency on the recurrence; with bufs=2 pools
            their slot-reuse WAR naturally delays them into PE's idle
            windows of stage s-2. pn is split into two half-bank tiles so
            the a/b column-half tail chains have no false cross-deps.
            """
            do0 = s < n_steps
            pzr0 = (
                ps_zr0.tile([128, bl], f32, tag="zr0", name=f"pzr0_{s}")
                if do0
                else None
            )
            pn_a = ps_n.tile([128, hb], f32, tag="na", name=f"pna_{s}")
            pn_b = ps_n.tile([128, hb], f32, tag="nb", name=f"pnb_{s}")
            if do0:
                xs = x_slice(s % t_steps)
                nc.tensor.matmul(pzr0, lhsT=wzr0x, rhs=xs, start=True, stop=False)
                nc.tensor.matmul(pn_a[0:H, :], lhsT=wn0x, rhs=xs[:, 0:hb],
                                 start=True, stop=False, skip_group_check=True)
                nc.tensor.matmul(pn_b[0:H, :], lhsT=wn0x, rhs=xs[:, hb:bl],
                                 start=True, stop=False, skip_group_check=True)
            return pzr0, (pn_a, pn_b)

        def stage(s, do0, do1, cur):
            """Emit one wavefront stage; returns the new state tile."""
            nonlocal S
            pzr0, (pn_a, pn_b) = cur

            # ---- PE: state-dependent gate matmuls. pzr1 first: sigma1
            # opens the chain (t1 is the long pole through DVE). ----
            pzr1 = None
            if do1:
                pzr1 = ps_zr1.tile([128, bl], f32, tag="zr1")
                nc.tensor.matmul(pzr1, lhsT=wzr1, rhs=S, start=True, stop=True)
            if do0:
                nc.tensor.matmul(pzr0, lhsT=wzr0h, rhs=S[0:H, :], start=False,
                                 stop=True)
            ph = ps_h.tile([128, bl], f32, tag="hn")
            if do0 and do1:
                nc.tensor.matmul(ph, lhsT=whn, rhs=S, start=True, stop=True)
            elif do0:
                nc.tensor.matmul(ph[H:128, :], lhsT=whn[0:H, 64:128],
                                 rhs=S[0:H, :], start=True, stop=True)
            else:
                nc.tensor.matmul(ph[0:H, :], lhsT=whn[H:128, 0:64],
                                 rhs=S[H:128, :], start=True, stop=True)
            if do1:
                nc.tensor.matmul(pn_a[H:128, :], lhsT=wn1x, rhs=S[0:H, 0:hb],
                                 start=True, stop=False, skip_group_check=True)
                nc.tensor.matmul(pn_b[H:128, :], lhsT=wn1x, rhs=S[0:H, hb:bl],
                                 start=True, stop=False, skip_group_check=True)

            # ---- ACT: sigmas (sigma1 first); zc = sigmoid(-x) later fills
            # ACT's idle window before the tanh halves ----
            rz0 = rz1 = None
            if do1:
                rz1 = work.tile([128, bl], bf16, tag="rz1")
                nc.scalar.activation(rz1, pzr1, AF.Sigmoid, bias=bzr1)
            if do0:
                rz0 = work.tile([128, bl], bf16, tag="rz0")
                nc.scalar.activation(rz0, pzr0, AF.Sigmoid, bias=bzr0)

            # t = (hn + bnh) * r, both on DVE (t1 full first, t0 in column
            # halves so ident/tanh column-halves can start early).
            tt = work.tile([128, bl], bf16, tag="t")
            zc = work.tile([128, bl], bf16, tag="zc")
            w = work.tile([128, bl], bf16, tag="w")
            if do1:
                nc.vector.scalar_tensor_tensor(
                    out=tt[0:H, :], in0=ph[0:H, :], scalar=bnh[0:H, :],
                    in1=rz1[0:H, :], op0=OP.add, op1=OP.mult)
                nc.scalar.activation(zc[H:128, :], pzr1[H:128, :], AF.Sigmoid,
                                     bias=bzneg[H:128, :], scale=-1.0)
                nc.gpsimd.tensor_mul(w[H:128, :], rz1[H:128, :], S[H:128, :])
            if do0:
                if do1:
                    nc.vector.scalar_tensor_tensor(
                        out=tt[H:128, 0:hb], in0=ph[H:128, 0:hb],
                        scalar=bnh[H:128, :], in1=rz0[H:128, 0:hb],
                        op0=OP.add, op1=OP.mult)
                    nc.vector.scalar_tensor_tensor(
                        out=tt[H:128, hb:bl], in0=ph[H:128, hb:bl],
                        scalar=bnh[H:128, :], in1=rz0[H:128, hb:bl],
                        op0=OP.add, op1=OP.mult)
                else:
                    nc.vector.scalar_tensor_tensor(
                        out=tt[H:128, :], in0=ph[H:128, :],
                        scalar=bnh[H:128, :], in1=rz0[H:128, :],
                        op0=OP.add, op1=OP.mult)
                nc.scalar.activation(zc[0:H, :], rz0[0:H, :], AF.Copy,
                                     bias=1.0, scale=-1.0)
                nc.gpsimd.tensor_mul(w[0:H, :], rz0[0:H, :], S[0:H, :])

            # ---- PE ident + ACT tanh + DVE u/ns, pipelined in column
            # halves so the tail overlaps itself ----
            nn = work.tile([128, bl], bf16, tag="n")
            u = work.tile([128, bl], bf16, tag="u")
            S_new = spool.tile([128, bl], bf16, tag="S")
            lo, hi = (0 if do0 else H), (128 if do1 else H)

            if do0 and do1:
                for pnh, c0, c1 in ((pn_a, 0, hb), (pn_b, hb, bl)):
                    nc.tensor.matmul(pnh, lhsT=idsw, rhs=tt[:, c0:c1],
                                     start=False, stop=True,
                                     skip_group_check=True)
                    nc.scalar.activation(nn[:, c0:c1], pnh, AF.Tanh, bias=bni)
                    nc.vector.tensor_mul(u[:, c0:c1], nn[:, c0:c1],
                                         zc[:, c0:c1])
                    nc.vector.tensor_add(S_new[:, c0:c1], u[:, c0:c1],
                                         w[:, c0:c1])
            else:
                for pnh, c0, c1 in ((pn_a, 0, hb), (pn_b, hb, bl)):
                    if do0:
                        nc.tensor.matmul(pnh[0:H, :], lhsT=idsw[H:128, 0:64],
                                         rhs=tt[H:128, c0:c1], start=False,
                                         stop=True, skip_group_check=True)
                    else:
                        nc.tensor.matmul(pnh[H:128, :], lhsT=idsw[0:H, 64:128],
                                         rhs=tt[0:H, c0:c1], start=False,
                                         stop=True, skip_group_check=True)
                    nc.scalar.activation(nn[lo:hi, c0:c1], pnh[lo:hi, :],
                                         AF.Tanh, bias=bni[lo:hi, :])
                    nc.vector.tensor_mul(u[lo:hi, c0:c1], nn[lo:hi, c0:c1],
                                         zc[lo:hi, c0:c1])
                    nc.vector.tensor_add(S_new[lo:hi, c0:c1], u[lo:hi, c0:c1],
                                         w[lo:hi, c0:c1])
                olo, ohi = (H, 128) if do0 else (0, H)
                nc.vector.tensor_copy(S_new[olo:ohi, :], S[olo:ohi, :])
            S = S_new

            # Emit stage s+2's x-matmuls at the stage BOTTOM. With bufs=2 on
            # their psum pools, slot-reuse WAR naturally delays them: the
            # zr x-matmul becomes ready when sigma0(s) frees its slot (PE's
            # idle sigma-window) and the n x-matmul when tanh(s) does (PE's
            # idle tail window) — so neither bypasses in front of the
            # chain-critical ident matmuls.
            if s + 2 <= n_steps:
                tiles[s + 2] = emit_xpart(s + 2)

        tiles = {0: emit_xpart(0), 1: emit_xpart(1)}
        for s in range(n_steps + 1):
            stage(s, do0=(s < n_steps), do1=(s >= 1), cur=tiles.pop(s))

        # final projection: out = fc_w @ h + fc_b   -> [1, bl]
        pfc = ps_zr0.tile([1, bl], f32, tag="zr0")
        nc.tensor.matmul(pfc, lhsT=fcw, rhs=S[H:128, :], start=True, stop=True)
        out_sb = work.tile([1, bl], f32, tag="out", bufs=1)
        nc.scalar.activation(out_sb, pfc, AF.Identity, bias=fcb)
        nc.sync.dma_start(out=out_d[:], in_=out_sb)

    _legalize_sync(nc, mybir)
    return nc


def shard_inputs(inputs, bl=BL, ncores=NCORES, t_steps=T):
    """Host-side prep: transpose/cast/shard full inputs into per-core maps."""
    bf = ml_dtypes.bfloat16
    x = np.asarray(inputs["x"], dtype=np.float32)
    # [B, T, D] -> [D, T, B]
    xT = np.ascontiguousarray(
        x[: bl * ncores, :t_steps, :].transpose(2, 1, 0)
    ).astype(bf)

    f32 = np.float32
    Wih0 = np.asarray(inputs["W_ih0"], f32)
    Whh0 = np.asarray(inputs["W_hh0"], f32)
    Wih1 = np.asarray(inputs["W_ih1"], f32)
    Whh1 = np.asarray(inputs["W_hh1"], f32)

    CW = 776
    cb = np.zeros((128, CW), dtype=bf)
    # layer0 gate order [z | r], layer1 mirrored [r | z]
    cb[0:D, 0:128] = np.concatenate([Wih0[H : 2 * H], Wih0[0:H]], axis=0).T.astype(bf)
    cb[0:H, 128:256] = np.concatenate([Whh0[H : 2 * H], Whh0[0:H]], axis=0).T.astype(bf)
    wzr1 = np.zeros((128, 128), f32)
    wzr1[0:H, :] = np.concatenate([Wih1[0:H], Wih1[H : 2 * H]], axis=0).T
    wzr1[H:128, :] = np.concatenate([Whh1[0:H], Whh1[H : 2 * H]], axis=0).T
    cb[:, 256:384] = wzr1.astype(bf)
    whn = np.zeros((128, 128), f32)
    whn[H:128, 0:H] = Whh1[2 * H :].T  # hn1 from h
    whn[0:H, H:128] = Whh0[2 * H :].T  # hn0 from g
    cb[:, 384:512] = whn.astype(bf)
    cb[0:D, 512:576] = Wih0[2 * H :].T.astype(bf)
    cb[0:H, 576:640] = Wih1[2 * H :].T.astype(bf)
    idsw = np.zeros((128, 128), f32)
    idsw[H:128, 0:H] = np.eye(H)  # out[0:64]  <- T[64:128]
    idsw[0:H, H:128] = np.eye(H)  # out[64:128] <- T[0:64]
    cb[:, 640:768] = idsw.astype(bf)
    cb[H:128, 768] = np.asarray(inputs["fc_w"], f32).reshape(H).astype(bf)

    cf = np.zeros((128, 8), dtype=f32)
    b0 = np.asarray(inputs["b_ih0"], f32) + np.asarray(inputs["b_hh0"], f32)
    b1 = np.asarray(inputs["b_ih1"], f32) + np.asarray(inputs["b_hh1"], f32)
    cf[:, 0] = np.concatenate([b0[H : 2 * H], b0[0:H]])  # [z0 | r0]
    cf[:, 1] = np.concatenate([b1[0:H], b1[H : 2 * H]])  # [r1 | z1]
    cf[0:H, 2] = np.asarray(inputs["b_ih0"], f32)[2 * H :]
    cf[H:128, 2] = np.asarray(inputs["b_ih1"], f32)[2 * H :]
    cf[0:H, 3] = np.asarray(inputs["b_hh1"], f32)[2 * H :]
    cf[H:128, 3] = np.asarray(inputs["b_hh0"], f32)[2 * H :]
    cf[0, 5] = np.asarray(inputs["fc_b"], f32).reshape(())
    cf[0:H, 6] = -b0[H : 2 * H]  # -z0 bias
    cf[H:128, 6] = -b1[H : 2 * H]  # -z1 bias

    shared = {"cb": cb, "cf": cf}

    in_maps = []
    for c in range(ncores):
        m = dict(shared)
        m["x"] = np.ascontiguousarray(xT[:, :, c * bl : (c + 1) * bl])
        in_maps.append(m)
    return in_maps


def kernel(**inputs):
    from concourse import bass_utils

    if "nc" not in _CACHE:
        _CACHE["nc"] = build_module()
    nc = _CACHE["nc"]
    in_maps = shard_inputs(inputs)
    res = bass_utils.run_bass_kernel_spmd(nc, in_maps, core_ids=list(range(NCORES)))
    out = np.concatenate([r["out"].reshape(BL) for r in res.results])
    return out.astype(np.float32)
